# revision 1
# baseline (speedup 1.0000x reference)
"""BilinearAttention Trainium2 kernel — 8-core data-parallel (batch sharded).

Math per batch element b (reference semantics):
  d   = drug @ Wd + bd                     (N=128, HID=512)
  dWb = drug @ (Wd@Wb folded) + bdWb       (N, HID)     [host-folded weights]
  t   = target @ Wt + bt                   (L=1024, HID)
  per head h (HD=64):
    attn = dWb_h @ t_h^T                   (N, L)
    E    = exp(attn)  with masked rows/cols exactly 0 in the *inputs*
    d-side: p_d = E / rowsum(E);  w_d[l] = sum_n p_d * dm[n]/dlen
            ctx_d[h] = sum_l w_d[l] * t_h[l]
    t-side: p_t = E / colsum(E);  w_t[n] = sum_l p_t * tm[l]/tlen
            ctx_t[h] = sum_n w_t[n] * d_h[n]
  out[b] = [ctx_d(512) | ctx_t(512)]

Mask scheme: host zeroes invalid rows of drug/target; projection biases are
applied via rank-1 (bias x mask01) matmuls so projected features are exactly
0 at invalid positions.  exp(0)=1 at invalid attn entries; softmax
denominators are fixed by subtracting the host-known invalid counts.

Schedule: software-pipelined across batch elements — the (PE-heavy)
projection of batch b+1 is interleaved into the (ACT-limited) exp phases of
batch b so the tensor engine never drains.
"""

import numpy as np
import ml_dtypes

import concourse.bass as bass
import concourse.bacc as bacc
import concourse.mybir as mybir
from concourse.bass_utils import run_bass_kernel_spmd
from concourse import tile
from concourse.masks import make_identity

NCORES = 8
B = 32
BC = B // NCORES          # 4 batch elements per core
N, L = 128, 1024
KD, KT = 256, 1280        # drug dim, target dim
HID, H, HD = 512, 8, 64
NKC_T = KT // 128         # 10 k-chunks for target proj
NKC_D = KD // 128         # 2 k-chunks for drug proj
NC4 = HID // 128          # 4 hid chunks (2 heads each)
FP32 = mybir.dt.float32
BF16 = mybir.dt.bfloat16
AF = mybir.ActivationFunctionType
ALU = mybir.AluOpType
BF16NP = ml_dtypes.bfloat16


def _body(tc, io):
    nc = tc.nc
    import contextlib
    es = contextlib.ExitStack()

    const = es.enter_context(tc.tile_pool(name="const", bufs=1))

    # ---- weights / constants: 4 coalesced DMAs ----
    wt_all = const.tile([128, NKC_T * HID], BF16, tag="wtall")
    nc.sync.dma_start(
        out=wt_all[:].rearrange("p (kc h) -> p kc h", h=HID),
        in_=io["wt"].rearrange("(kc p) h -> p kc h", p=128))
    wd2_all = const.tile([128, NKC_D * 2 * HID], BF16, tag="wd2all")
    nc.sync.dma_start(
        out=wd2_all[:].rearrange("p (kc h) -> p kc h", h=2 * HID),
        in_=io["wd2"].rearrange("(kc p) h -> p kc h", p=128))
    # cpack fp32 [128, 3*BC + BC*64]: dmwT | sdcorrT | stcorrT | tmw64 per b
    cpack = const.tile([128, 3 * BC + BC * 64], FP32, tag="cpack")
    nc.sync.dma_start(out=cpack[:], in_=io["cpack"][:])
    dmwT = cpack[:, 0:BC]
    sdcorrT = cpack[:, BC:2 * BC]
    stcorrT = cpack[:, 2 * BC:3 * BC]
    tmw64_t = [cpack[:, 3 * BC + b * 64:3 * BC + (b + 1) * 64]
               for b in range(BC)]
    # rpack bf16 [1, 512 + 1024 + BC*1024 + BC*128]: bt | b2 | tmask | dmask
    rpack = const.tile([1, HID + 2 * HID + BC * L + BC * N], BF16, tag="rpack")
    nc.sync.dma_start(out=rpack[:], in_=io["rpack"][:])
    bt_row = rpack[:, 0:HID]
    b2_row = rpack[:, HID:HID + 2 * HID]
    tmask_row_t = [rpack[:, 3 * HID + b * L:3 * HID + (b + 1) * L]
                   for b in range(BC)]
    dmask_row_t = [rpack[:, 3 * HID + BC * L + b * N:
                         3 * HID + BC * L + (b + 1) * N]
                   for b in range(BC)]

    ident_f = const.tile([128, 128], FP32, tag="idf")
    make_identity(nc, ident_f[:])
    ones64 = const.tile([128, 64], BF16, tag="ones64")
    nc.vector.memset(ones64[:], 1.0)
    ctxT_all = const.tile([128, 128], FP32, tag="ctxall")

    # ---- pools ----
    tgtT_pool = es.enter_context(tc.tile_pool(name="tgtT", bufs=2))
    tT_pool = es.enter_context(tc.tile_pool(name="tT", bufs=2))
    e_pool = es.enter_context(tc.tile_pool(name="E", bufs=12))
    et_pool = es.enter_context(tc.tile_pool(name="Et", bufs=12))
    d2T_pool = es.enter_context(tc.tile_pool(name="d2T", bufs=2))
    casc_pool = es.enter_context(tc.tile_pool(name="casc", bufs=2))
    small = es.enter_context(tc.tile_pool(name="small", bufs=4))
    junk = es.enter_context(tc.tile_pool(name="junk", bufs=3))
    ps2 = es.enter_context(tc.tile_pool(name="ps2", bufs=3, space="PSUM"))
    ps1 = es.enter_context(tc.tile_pool(name="ps1", bufs=2, space="PSUM"))

    # ---------- helpers ----------
    def load_tgtT(bb):
        t = tgtT_pool.tile([128, NKC_T * L], BF16, tag="tgtT",
                           name=f"tgtT_{bb}")
        for half in range(2):
            nc.sync.dma_start(
                out=t[:].rearrange("p (kc l) -> p kc l", l=L)[
                    :, half * 5:(half + 1) * 5, :],
                in_=io["tgtT"][bb, half * 5 * 128:(half + 1) * 5 * 128, :]
                .rearrange("(kc p) l -> p kc l", p=128),
            )
        return t

    def load_drugT(bb):
        t = d2T_pool.tile([128, NKC_D * N], BF16, tag="drugT",
                          name=f"drugT_{bb}")
        nc.sync.dma_start(
            out=t[:].rearrange("p (kc n) -> p kc n", n=N),
            in_=io["drug_bf"][bb].rearrange("(kc p) n -> p kc n", p=128),
        )
        return t

    tproj_state = {}

    def tproj_group(bb, tgtT_t, slot):
        # slot 0..7 -> (c, lh); psum tile held across even/odd slot pairs
        c, lh = slot // 2, slot % 2
        if lh == 0:
            tproj_state["ps"] = ps2.tile([128, 2 * 512], FP32, tag="ps2",
                                         name=f"ps_tp_{bb}_{c}")
        ps = tproj_state["ps"]
        for kc in range(NKC_T):
            nc.tensor.matmul(
                ps[:, lh * 512:(lh + 1) * 512],
                lhsT=wt_all[:, kc * HID + c * 128:kc * HID + (c + 1) * 128],
                rhs=tgtT_t[:, kc * L + lh * 512:kc * L + (lh + 1) * 512],
                start=(kc == 0), stop=False,
            )
        nc.tensor.matmul(
            ps[:, lh * 512:(lh + 1) * 512],
            lhsT=bt_row[:, c * 128:(c + 1) * 128],
            rhs=tmask_row_t[bb][:, lh * 512:(lh + 1) * 512],
            start=False, stop=True,
        )
        if lh == 1:
            t = tT_pool.tile([128, L], BF16, tag=f"tT{c}", name=f"tT_{bb}_{c}")
            nc.scalar.copy(t[:], ps[:])
            return t
        return None

    def dproj(bb, drugT_t):
        d2T = d2T_pool.tile([128, 8 * N], BF16, tag="d2T", name=f"d2T_{bb}")
        ps_d = ps2.tile([128, 2 * 512], FP32, tag="ps2", name=f"ps_dp_{bb}")
        for ch in range(8):
            for kc in range(NKC_D):
                nc.tensor.matmul(
                    ps_d[:, ch * 128:(ch + 1) * 128],
                    lhsT=wd2_all[:, kc * 2 * HID + ch * 128:
                                 kc * 2 * HID + (ch + 1) * 128],
                    rhs=drugT_t[:, kc * N:(kc + 1) * N],
                    start=(kc == 0), stop=False,
                )
            nc.tensor.matmul(
                ps_d[:, ch * 128:(ch + 1) * 128],
                lhsT=b2_row[:, ch * 128:(ch + 1) * 128],
                rhs=dmask_row_t[bb][:],
                start=False, stop=True,
            )
        d2T_new = d2T
        nc.scalar.copy(d2T_new[:], ps_d[:])
        # zero-padded dW halves for the K=128 Et matmuls
        d2pair = d2T_pool.tile([128, 2 * 512], BF16, tag="d2pair",
                               name=f"d2pair_{bb}")
        nc.vector.memset(d2pair[:], 0.0)
        nc.vector.tensor_copy(d2pair[0:64, 0:512], d2T_new[0:64, 512:1024])
        nc.vector.tensor_copy(d2pair[64:128, 512:1024],
                              d2T_new[64:128, 512:1024])
        return d2T_new, d2pair

    # ---------- prologue: batch 0 projections ----------
    tgtT_cur = load_tgtT(0)
    drugT_cur = load_drugT(0)
    tT_cur = []
    for slot in range(8):
        t = tproj_group(0, tgtT_cur, slot)
        if t is not None:
            tT_cur.append(t)
    d2T_cur, d2pair_cur = dproj(0, drugT_cur)

    for b in range(BC):
        nxt = b + 1 if b + 1 < BC else None
        if nxt is not None:
            tgtT_nxt = load_tgtT(nxt)
            drugT_nxt = load_drugT(nxt)
        tT, d2T, d2pair = tT_cur, d2T_cur, d2pair_cur

        # ---- E phase (ACT-limited) interleaved with t-proj(b+1) ----
        E = [e_pool.tile([128, L], BF16, tag="E", name=f"E_{b}_{i}")
             for i in range(H)]
        S_d8 = small.tile([128, 8], FP32, tag="Sd8")
        tT_nxt = []
        for h in range(H):
            c, ph = h // 2, (h % 2) * 64
            ps = ps2.tile([128, 2 * 512], FP32, tag="ps2",
                          name=f"ps_E_{b}_{h}")
            for lh in range(2):
                nc.tensor.matmul(
                    ps[:, lh * 512:(lh + 1) * 512],
                    lhsT=d2T[ph:ph + 64, (4 + c) * 128:(5 + c) * 128],
                    rhs=tT[c][ph:ph + 64, lh * 512:(lh + 1) * 512],
                    start=True, stop=True,
                )
            nc.scalar.activation(
                E[h][:], ps[:], AF.Exp,
                accum_out=S_d8[:, h:h + 1],
            )
            if nxt is not None:
                t = tproj_group(nxt, tgtT_nxt, h)
                if t is not None:
                    tT_nxt.append(t)

        # ---- u pipeline (DVE) — overlaps the Et matmuls below ----
        nc.vector.tensor_scalar(
            out=S_d8[:], in0=S_d8[:], scalar1=sdcorrT[:, b:b + 1],
            scalar2=None, op0=ALU.add,
        )
        recipSd = small.tile([128, 8], FP32, tag="rSd")
        nc.vector.reciprocal(recipSd[:], S_d8[:])
        u_f = small.tile([128, 8], FP32, tag="uf")
        nc.vector.tensor_scalar(
            out=u_f[:], in0=recipSd[:], scalar1=dmwT[:, b:b + 1],
            scalar2=None, op0=ALU.mult,
        )
        u_rep = small.tile([128, 8 * 64], BF16, tag="urep")
        for h in range(H):
            nc.vector.tensor_scalar(
                out=u_rep[:, h * 64:(h + 1) * 64], in0=ones64[:],
                scalar1=u_f[:, h:h + 1], scalar2=None, op0=ALU.mult,
            )

        # ---- Et phase (ACT-limited); cascade on DVE per lc ----
        Et = [et_pool.tile([128, 8 * N], BF16, tag="Et", name=f"Et_{b}_{i}")
              for i in range(8)]
        S_t = small.tile([128, 64], FP32, tag="St")
        for lc in range(8):
            ps = ps2.tile([128, 2 * 512], FP32, tag="ps2",
                          name=f"ps_Et_{b}_{lc}")
            for hh in range(8):
                c, hp = hh // 2, hh % 2
                nc.tensor.matmul(
                    ps[:, hh * 128:(hh + 1) * 128],
                    lhsT=tT[c][:, lc * 128:(lc + 1) * 128],
                    rhs=d2pair[:, hp * 512 + c * 128:hp * 512 + (c + 1) * 128],
                    start=True, stop=True,
                )
            nc.scalar.activation(Et[lc][:], ps[:], AF.Exp)
            t1 = casc_pool.tile([128, 512], BF16, tag="t1")
            v = Et[lc][:].rearrange("p (s n) -> p s n", n=128)
            nc.vector.tensor_tensor(
                t1[:].rearrange("p (s n) -> p s n", n=64),
                v[:, :, 0:64], v[:, :, 64:128], ALU.add)
            t2 = casc_pool.tile([128, 256], BF16, tag="t2")
            v = t1[:].rearrange("p (s n) -> p s n", n=64)
            nc.vector.tensor_tensor(
                t2[:].rearrange("p (s n) -> p s n", n=32),
                v[:, :, 0:32], v[:, :, 32:64], ALU.add)
            nc.vector.tensor_reduce(
                S_t[:, lc * 8:(lc + 1) * 8],
                t2[:].rearrange("p (s n) -> p s n", n=32),
                axis=mybir.AxisListType.X, op=ALU.add,
            )

        # ---- d-proj(b+1) fills the Et-phase PE tail ----
        if nxt is not None:
            d2T_nxt, d2pair_nxt = dproj(nxt, drugT_nxt)

        # ---- g (DVE) ----
        nc.vector.tensor_scalar(
            out=S_t[:], in0=S_t[:], scalar1=stcorrT[:, b:b + 1],
            scalar2=None, op0=ALU.add,
        )
        recipSt = small.tile([128, 64], FP32, tag="rSt")
        nc.vector.reciprocal(recipSt[:], S_t[:])
        g_f = small.tile([128, 64], FP32, tag="gf")
        nc.vector.tensor_tensor(g_f[:], recipSt[:], tmw64_t[b][:], ALU.mult)

        # ---- w_d phase (PE) — overlaps the g-fold on DVE ----
        ctxv = small.tile([128, 8], FP32, tag="ctx")
        for c in range(NC4):
            ps = ps2.tile([128, 2 * 512], FP32, tag="ps2",
                          name=f"ps_wd_{b}_{c}")
            for lh in range(2):
                for hp in range(2):
                    h = 2 * c + hp
                    nc.tensor.matmul(
                        ps[hp * 64:(hp + 1) * 64, lh * 512:(lh + 1) * 512],
                        lhsT=u_rep[:, h * 64:(h + 1) * 64],
                        rhs=E[h][:, lh * 512:(lh + 1) * 512],
                        start=True, stop=True,
                    )
            scratch = junk.tile([128, 1024], BF16, tag="junk")
            nc.vector.scalar_tensor_tensor(
                out=scratch[:], in0=ps[:], scalar=1.0,
                in1=tT[c][:],
                op0=ALU.mult, op1=ALU.mult,
                accum_out=ctxv[:, c:c + 1],
            )

        # fold g into Et rows (broadcast per head segment), in place
        for lc in range(8):
            g_b = g_f[:, lc * 8:(lc + 1) * 8, None].to_broadcast((128, 8, 128))
            nc.vector.tensor_tensor(
                Et[lc][:].rearrange("p (h n) -> p h n", h=8),
                Et[lc][:].rearrange("p (h n) -> p h n", h=8),
                g_b, ALU.mult,
            )
        # w_t replicated via ones-stationary: ps_wt[(h%2)*64+e, n] = w_t[h, n]
        ps_wt = ps1.tile([128, 512], FP32, tag="ps1", name=f"ps_wt_{b}")
        for h in range(H):
            c, ph = h // 2, (h % 2) * 64
            for lc in range(8):
                nc.tensor.matmul(
                    ps_wt[ph:ph + 64, c * 128:(c + 1) * 128],
                    lhsT=ones64[:],
                    rhs=Et[lc][:, h * 128:(h + 1) * 128],
                    start=(lc == 0), stop=(lc == 7),
                )
        for c in range(NC4):
            scratch = junk.tile([128, 1024], BF16, tag="junk")
            nc.vector.scalar_tensor_tensor(
                out=scratch[:, 0:128], in0=ps_wt[:, c * 128:(c + 1) * 128],
                scalar=1.0,
                in1=d2T[:, c * 128:(c + 1) * 128],
                op0=ALU.mult, op1=ALU.mult,
                accum_out=ctxv[:, 4 + c:5 + c],
            )

        # ---------- transpose ctx [128, 8] -> [8, 128] and stage ----------
        ps_c = ps1.tile([128, 512], FP32, tag="ps1", name=f"ps_c_{b}")
        nc.tensor.transpose(ps_c[0:8, 0:128], ctxv[:], ident_f[:])
        nc.scalar.copy(ctxT_all[b * 32:b * 32 + 8, :], ps_c[0:8, 0:128])

        if nxt is not None:
            tT_cur, d2T_cur, d2pair_cur = tT_nxt, d2T_nxt, d2pair_nxt

    # ---------- output DMA: [32, 128] -> (BC, 1024) ----------
    for b in range(BC):
        nc.sync.dma_start(
            out=io["out"][b].rearrange("(j p) -> j p", j=8),
            in_=ctxT_all[b * 32:b * 32 + 8, :],
        )
    es.close()


def _build():
    nc = bacc.Bacc("TRN2", target_bir_lowering=False, debug=False,
                   num_devices=NCORES)
    io = {}

    def inp(name, shape, dt):
        io[name] = nc.dram_tensor(name, shape, dt, kind="ExternalInput").ap()

    inp("tgtT", [BC, KT, L], BF16)
    inp("drug_bf", [BC, KD, N], BF16)
    inp("wt", [KT, HID], BF16)
    inp("wd2", [KD, 2 * HID], BF16)
    inp("cpack", [128, 3 * BC + BC * 64], FP32)
    inp("rpack", [1, HID + 2 * HID + BC * L + BC * N], BF16)
    io["out"] = nc.dram_tensor("out", [BC, 2 * HID], FP32,
                               kind="ExternalOutput").ap()
    with tile.TileContext(nc) as tc:
        _body(tc, io)
    nc.compile()
    return nc


_NC_CACHE = None
_LAST_RESULTS = None


def _prep_host(drug_nodes, drug_mask, target_seq, target_mask,
               Wd, bd, Wt, bt, Wb):
    f32 = np.float32
    WdWb = np.einsum("khd,hde->khe", Wd.reshape(KD, H, HD), Wb).reshape(KD, HID)
    bdWb = np.einsum("hd,hde->he", bd.reshape(H, HD), Wb).reshape(HID)
    wd2 = np.ascontiguousarray(
        np.concatenate([Wd, WdWb], axis=1)).astype(BF16NP)
    wt_bf = np.ascontiguousarray(Wt).astype(BF16NP)
    b2 = np.concatenate([bd, bdWb]).astype(f32)
    dlen = np.maximum(drug_mask.sum(-1), 1).astype(f32)
    tlen = np.maximum(target_mask.sum(-1), 1).astype(f32)
    dmw = (drug_mask.astype(f32) / dlen[:, None]).astype(f32)
    tmw = (target_mask.astype(f32) / tlen[:, None]).astype(f32)
    sdcorr = (1e-30 - (L - tlen)).astype(f32)      # subtract invalid-l count
    stcorr = (1e-30 - (N - dlen)).astype(f32)      # subtract invalid-n count
    return wd2, wt_bf, b2, dmw, tmw, sdcorr, stcorr


def kernel(drug_nodes, drug_mask, target_seq, target_mask,
           Wd, bd, Wt, bt, Wb):
    f32 = np.float32
    drug_nodes = np.asarray(drug_nodes, f32)
    drug_mask = np.asarray(drug_mask)
    target_seq = np.asarray(target_seq, f32)
    target_mask = np.asarray(target_mask)
    Wd, bd = np.asarray(Wd, f32), np.asarray(bd, f32)
    Wt, bt = np.asarray(Wt, f32), np.asarray(bt, f32)
    Wb = np.asarray(Wb, f32)

    (wd2, wt_bf, b2, dmw, tmw, sdcorr, stcorr) = _prep_host(
        drug_nodes, drug_mask, target_seq, target_mask, Wd, bd, Wt, bt, Wb)

    tgt_bf = (target_seq * target_mask[:, :, None]).astype(BF16NP)
    tgtT_h = np.ascontiguousarray(tgt_bf.transpose(0, 2, 1))
    drug_bf = (drug_nodes * drug_mask[:, :, None]).astype(BF16NP)
    drugT_h = np.ascontiguousarray(drug_bf.transpose(0, 2, 1))
    bt_bf = bt.astype(BF16NP)
    b2_bf = b2.astype(BF16NP)
    tmask_bf = target_mask.astype(f32).astype(BF16NP)
    dmask_bf = drug_mask.astype(f32).astype(BF16NP)
    tmw64 = np.repeat(
        tmw.reshape(B, 8, 128).transpose(0, 2, 1), 8, axis=2)

    in_maps = []
    for i in range(NCORES):
        s = slice(i * BC, (i + 1) * BC)
        cpack = np.empty((128, 3 * BC + BC * 64), f32)
        cpack[:, 0:BC] = dmw[s].T
        cpack[:, BC:2 * BC] = sdcorr[s][None, :]
        cpack[:, 2 * BC:3 * BC] = stcorr[s][None, :]
        cpack[:, 3 * BC:] = tmw64[s].transpose(1, 0, 2).reshape(128, BC * 64)
        rpack = np.concatenate(
            [bt_bf, b2_bf, tmask_bf[s].ravel(), dmask_bf[s].ravel()])[None, :]
        in_maps.append(dict(
            tgtT=np.ascontiguousarray(tgtT_h[s]),
            drug_bf=np.ascontiguousarray(drugT_h[s]),
            wt=wt_bf, wd2=wd2,
            cpack=np.ascontiguousarray(cpack),
            rpack=np.ascontiguousarray(rpack),
        ))

    nc = _get_nc()
    res = run_bass_kernel_spmd(nc, in_maps, list(range(NCORES)))
    global _LAST_RESULTS
    _LAST_RESULTS = res
    out = np.concatenate([res.results[i]["out"] for i in range(NCORES)],
                         axis=0)
    return np.ascontiguousarray(out.astype(np.float32))


def _get_nc():
    global _NC_CACHE
    if _NC_CACHE is None:
        _NC_CACHE = _build()
    return _NC_CACHE



# revision 10
# speedup vs baseline: 1.0724x; 1.0724x over previous
"""BilinearAttention Trainium2 kernel — 8-core data-parallel (batch sharded).

Math per batch element b (reference semantics):
  d   = drug @ Wd + bd                     (N=128, HID=512)
  dWb = drug @ (Wd@Wb folded) + bdWb       (N, HID)     [host-folded weights]
  t   = target @ Wt + bt                   (L=1024, HID)
  per head h (HD=64):
    attn = dWb_h @ t_h^T                   (N, L)
    E    = exp(attn)  with masked rows/cols exactly 0 in the *inputs*
    d-side: p_d = E / rowsum(E);  w_d[l] = sum_n p_d * dm[n]/dlen
            ctx_d[h] = sum_l w_d[l] * t_h[l]
    t-side: p_t = E / colsum(E);  w_t[n] = sum_l p_t * tm[l]/tlen
            ctx_t[h] = sum_n w_t[n] * d_h[n]
  out[b] = [ctx_d(512) | ctx_t(512)]

Mask scheme: host zeroes invalid rows of drug/target; projection biases are
applied via rank-1 (bias x mask01) matmuls so projected features are exactly
0 at invalid positions.  exp(0)=1 at invalid attn entries; softmax
denominators are fixed by subtracting the host-known invalid counts.

Schedule notes:
 - prologue: drug projection for ALL 4 batch elements in one batched GEMM
   (starts ~2us in, warms the PE while tgtT(0) streams), then tproj(0)
   paced by per-k-chunk DMAs.
 - steady state: tproj(b+1) interleaved into the (ACT-limited) E phase of
   batch b so the tensor engine never drains.
 - t-side: attn^T recomputed per 128-row l-chunk with a head-pair repacked
   rhs (free=256 matmuls); the per-(l,head) softmax scale g = tmw/colsum is
   folded into the w_t reduction as the matmul *stationary* operand (gdual:
   [g_h | g_h'] column-doubled), so no broadcast multiply over Et is needed.
   The resulting PSUM has garbage in the cross quadrants, which are never
   read.
"""

import numpy as np
import ml_dtypes

import concourse.bass as bass
import concourse.bacc as bacc
import concourse.mybir as mybir
from concourse.bass_utils import run_bass_kernel_spmd
from concourse import tile
from concourse.masks import make_identity

NCORES = 8
B = 32
BC = B // NCORES          # 4 batch elements per core
N, L = 128, 1024
KD, KT = 256, 1280        # drug dim, target dim
HID, H, HD = 512, 8, 64
NKC_T = KT // 128         # 10 k-chunks for target proj
NKC_D = KD // 128         # 2 k-chunks for drug proj
NC4 = HID // 128          # 4 hid chunks (2 heads each)
FP32 = mybir.dt.float32
BF16 = mybir.dt.bfloat16
AF = mybir.ActivationFunctionType
ALU = mybir.AluOpType
BF16NP = ml_dtypes.bfloat16


def _body(tc, io):
    nc = tc.nc
    import contextlib
    es = contextlib.ExitStack()

    const = es.enter_context(tc.tile_pool(name="const", bufs=1))

    # ---- constants / packed scalars (small, go first) ----
    # cpack fp32 [128, 3*BC + BC*64]: dmwT | sdcorrT | stcorrT | tmw64 per b
    cpack = const.tile([128, 3 * BC + BC * 64], FP32, tag="cpack")
    nc.sync.dma_start(out=cpack[:], in_=io["cpack"][:])
    dmwT = cpack[:, 0:BC]
    sdcorrT = cpack[:, BC:2 * BC]
    stcorrT = cpack[:, 2 * BC:3 * BC]
    tmw64_t = [cpack[:, 3 * BC + b * 64:3 * BC + (b + 1) * 64]
               for b in range(BC)]
    # rpack bf16 [1, 512 + 1024 + BC*1024 + BC*128]: bt | b2 | tmask | dmask
    rpack = const.tile([1, HID + 2 * HID + BC * L + BC * N], BF16, tag="rpack")
    nc.sync.dma_start(out=rpack[:], in_=io["rpack"][:])
    bt_row = rpack[:, 0:HID]
    b2_row = rpack[:, HID:HID + 2 * HID]
    tmask_row_t = [rpack[:, 3 * HID + b * L:3 * HID + (b + 1) * L]
                   for b in range(BC)]
    dmask_all = rpack[:, 3 * HID + BC * L:3 * HID + BC * L + BC * N]

    # drug (all 4 batches, [kd-part, (kc, b, n)]) + wd2 weights: small DMAs,
    # issued first so the drug projection can start ~2us in.
    drugT_all = const.tile([128, NKC_D * BC * N], BF16, tag="drugT")
    for kc in range(NKC_D):
        nc.sync.dma_start(
            out=drugT_all[:, kc * BC * N:(kc + 1) * BC * N]
            .rearrange("p (b n) -> p b n", n=N),
            in_=io["drug_bf"][:, kc * 128:(kc + 1) * 128, :]
            .rearrange("b p n -> p b n"),
        )
    wd2_all = const.tile([128, NKC_D * 2 * HID], BF16, tag="wd2all")
    nc.sync.dma_start(
        out=wd2_all[:].rearrange("p (kc h) -> p kc h", h=2 * HID),
        in_=io["wd2"].rearrange("(kc p) h -> p kc h", p=128))

    # target-proj weights: per-k-chunk DMAs so tproj(0) can stream
    wt_all = const.tile([128, NKC_T * HID], BF16, tag="wtall")
    for kc in range(NKC_T):
        nc.sync.dma_start(
            out=wt_all[:, kc * HID:(kc + 1) * HID],
            in_=io["wt"][kc * 128:(kc + 1) * 128, :])

    ident_f = const.tile([128, 128], FP32, tag="idf")
    make_identity(nc, ident_f[:])
    ctxT_all = const.tile([128, 128], FP32, tag="ctxall")

    # d projections for all batches: d2T_all[:, ch*512 + b*128 + n]
    # ch 0..3 = d (value side), ch 4..7 = dW (bilinear-mapped, attn side)
    d2T_all = const.tile([128, 8 * BC * N], BF16, tag="d2Tall")
    # head-pair repacked dW with zero padding, per batch:
    # d2pair2[:, b*1024 + c*256 + s*128 + n]; valid rows s*64..s*64+63
    d2p2 = const.tile([128, BC * 2 * HID], BF16, tag="d2p2")

    # ---- pools ----
    tgtT_pool = es.enter_context(tc.tile_pool(name="tgtT", bufs=2))
    tT_pool = es.enter_context(tc.tile_pool(name="tT", bufs=2))
    e_pool = es.enter_context(tc.tile_pool(name="E", bufs=10))
    et_pool = es.enter_context(tc.tile_pool(name="Et", bufs=9))
    small = es.enter_context(tc.tile_pool(name="small", bufs=4))
    gd_pool = es.enter_context(tc.tile_pool(name="gd", bufs=9))
    junk = es.enter_context(tc.tile_pool(name="junk", bufs=3))
    ps_main = es.enter_context(tc.tile_pool(name="psm", bufs=3, space="PSUM"))
    ps_acc = es.enter_context(tc.tile_pool(name="psa", bufs=1, space="PSUM"))

    # ---------- helpers ----------
    def load_tgtT(bb, chunked=False):
        t = tgtT_pool.tile([128, NKC_T * L], BF16, tag="tgtT",
                           name=f"tgtT_{bb}")
        if chunked:
            for kc in range(NKC_T):
                nc.sync.dma_start(
                    out=t[:, kc * L:(kc + 1) * L],
                    in_=io["tgtT"][bb, kc * 128:(kc + 1) * 128, :])
        else:
            for half in range(2):
                nc.sync.dma_start(
                    out=t[:].rearrange("p (kc l) -> p kc l", l=L)[
                        :, half * 5:(half + 1) * 5, :],
                    in_=io["tgtT"][bb, half * 5 * 128:(half + 1) * 5 * 128, :]
                    .rearrange("(kc p) l -> p kc l", p=128),
                )
        return t

    tproj_state = {}

    def tproj_group(bb, tgtT_t, slot):
        # slot 0..7 -> (c, lh); psum tile held across even/odd slot pairs
        c, lh = slot // 2, slot % 2
        if lh == 0:
            tproj_state["ps"] = ps_main.tile([128, 2 * 512], FP32, tag="psm",
                                             name=f"ps_tp_{bb}_{c}")
        ps = tproj_state["ps"]
        for kc in range(NKC_T):
            nc.tensor.matmul(
                ps[:, lh * 512:(lh + 1) * 512],
                lhsT=wt_all[:, kc * HID + c * 128:kc * HID + (c + 1) * 128],
                rhs=tgtT_t[:, kc * L + lh * 512:kc * L + (lh + 1) * 512],
                start=(kc == 0), stop=False,
            )
        nc.tensor.matmul(
            ps[:, lh * 512:(lh + 1) * 512],
            lhsT=bt_row[:, c * 128:(c + 1) * 128],
            rhs=tmask_row_t[bb][:, lh * 512:(lh + 1) * 512],
            start=False, stop=True,
        )
        if lh == 1:
            t = tT_pool.tile([128, L], BF16, tag=f"tT{c}", name=f"tT_{bb}_{c}")
            nc.scalar.copy(t[:], ps[:])
            return t
        return None

    # ---------- prologue ----------
    # drug projection, all 4 batches in one batched GEMM (free dim = b*n=512)
    for cp in range(4):            # ch pairs (0,1), (2,3), (4,5), (6,7)
        ps_d = ps_main.tile([128, 2 * 512], FP32, tag="psm",
                            name=f"ps_dp_{cp}")
        for half in range(2):
            ch = 2 * cp + half
            for kc in range(NKC_D):
                nc.tensor.matmul(
                    ps_d[:, half * 512:(half + 1) * 512],
                    lhsT=wd2_all[:, kc * 2 * HID + ch * 128:
                                 kc * 2 * HID + (ch + 1) * 128],
                    rhs=drugT_all[:, kc * 512:(kc + 1) * 512],
                    start=(kc == 0), stop=False,
                )
            nc.tensor.matmul(
                ps_d[:, half * 512:(half + 1) * 512],
                lhsT=b2_row[:, ch * 128:(ch + 1) * 128],
                rhs=dmask_all[:],
                start=False, stop=True,
            )
        nc.scalar.copy(d2T_all[:, cp * 1024:(cp + 1) * 1024], ps_d[:])

    # head-pair repacked dW (zero padded halves)
    nc.vector.memset(d2p2[:], 0.0)
    for s in range(2):
        for bb in range(BC):
            nc.vector.tensor_copy(
                d2p2[s * 64:(s + 1) * 64, bb * 1024:(bb + 1) * 1024]
                .rearrange("q (c z) -> q c z", z=256)
                [:, :, s * 128:(s + 1) * 128],
                d2T_all[s * 64:(s + 1) * 64, 4 * BC * N:8 * BC * N]
                .rearrange("q (c z) -> q c z", z=512)
                [:, :, bb * 128:(bb + 1) * 128],
            )

    # tproj(0): streamed against the per-chunk DMAs
    tgtT_cur = load_tgtT(0, chunked=True)
    tT_cur = []
    for slot in range(8):
        t = tproj_group(0, tgtT_cur, slot)
        if t is not None:
            tT_cur.append(t)

    # ---------- per-batch steady state ----------
    for b in range(BC):
        nxt = b + 1 if b + 1 < BC else None
        if nxt is not None:
            tgtT_nxt = load_tgtT(nxt)
        tT = tT_cur

        # ---- E phase (ACT-limited) interleaved with t-proj(b+1) ----
        E = [e_pool.tile([128, L], BF16, tag="E", name=f"E_{b}_{i}")
             for i in range(H)]
        S_d8 = small.tile([128, 8], FP32, tag="Sd8")
        tT_nxt = []
        for h in range(H):
            c, ph = h // 2, (h % 2) * 64
            ps = ps_main.tile([128, 2 * 512], FP32, tag="psm",
                              name=f"ps_E_{b}_{h}")
            for lh in range(2):
                nc.tensor.matmul(
                    ps[:, lh * 512:(lh + 1) * 512],
                    lhsT=d2T_all[ph:ph + 64,
                                 (4 + c) * 512 + b * 128:(4 + c) * 512 + (b + 1) * 128],
                    rhs=tT[c][ph:ph + 64, lh * 512:(lh + 1) * 512],
                    start=True, stop=True,
                )
            nc.scalar.activation(
                E[h][:], ps[:], AF.Exp,
                accum_out=S_d8[:, h:h + 1],
            )
            if nxt is not None:
                t = tproj_group(nxt, tgtT_nxt, h)
                if t is not None:
                    tT_nxt.append(t)

        # ---- u pipeline (DVE) ----
        nc.vector.tensor_scalar(
            out=S_d8[:], in0=S_d8[:], scalar1=sdcorrT[:, b:b + 1],
            scalar2=None, op0=ALU.add,
        )
        recipSd = small.tile([128, 8], FP32, tag="rSd")
        nc.vector.reciprocal(recipSd[:], S_d8[:])
        u_f = small.tile([128, 8], FP32, tag="uf")
        nc.vector.tensor_scalar(
            out=u_f[:], in0=recipSd[:], scalar1=dmwT[:, b:b + 1],
            scalar2=None, op0=ALU.mult,
        )
        u_rep = small.tile([128, 8 * 64], BF16, tag="urep")
        nc.vector.tensor_copy(
            u_rep[:].rearrange("p (h z) -> p h z", z=64),
            u_f[:, :, None].to_broadcast((128, 8, 64)),
        )

        # ---- w_d phase (PE; folds on DVE) ----
        ctxv = small.tile([128, 8], FP32, tag="ctx")
        for c in range(NC4):
            ps = ps_main.tile([128, 2 * 512], FP32, tag="psm",
                              name=f"ps_wd_{b}_{c}")
            for lh in range(2):
                for hp in range(2):
                    h = 2 * c + hp
                    nc.tensor.matmul(
                        ps[hp * 64:(hp + 1) * 64, lh * 512:(lh + 1) * 512],
                        lhsT=u_rep[:, h * 64:(h + 1) * 64],
                        rhs=E[h][:, lh * 512:(lh + 1) * 512],
                        start=True, stop=True,
                    )
            scratch = junk.tile([128, 1024], BF16, tag="junk")
            nc.vector.scalar_tensor_tensor(
                out=scratch[:], in0=ps[:], scalar=1.0,
                in1=tT[c][:],
                op0=ALU.mult, op1=ALU.mult,
                accum_out=ctxv[:, c:c + 1],
            )

        # ---- Et phase: attn^T per l-chunk; g folded into the w_t matmul ----
        S_t = small.tile([128, 64], FP32, tag="St")
        ps_wt = ps_acc.tile([128, 1024], FP32, tag="psa", name=f"ps_wt_{b}")
        Et_tiles = []
        for lc in range(8):
            ps = ps_main.tile([128, 2 * 512], FP32, tag="psm",
                              name=f"ps_Et_{b}_{lc}")
            for c in range(NC4):
                nc.tensor.matmul(
                    ps[:, c * 256:(c + 1) * 256],
                    lhsT=tT[c][:, lc * 128:(lc + 1) * 128],
                    rhs=d2p2[:, b * 1024 + c * 256:b * 1024 + (c + 1) * 256],
                    start=True, stop=True,
                )
            Et = et_pool.tile([128, 1024], BF16, tag="Et", name=f"Et_{b}_{lc}")
            nc.scalar.activation(Et[:], ps[:], AF.Exp)
            # colsum cascade (over n, within each head's 128-block)
            t1 = junk.tile([128, 1024], BF16, tag="junk", name=f"t1_{b}_{lc}")
            v = Et[:].rearrange("p (s n) -> p s n", n=128)
            nc.vector.tensor_tensor(
                t1[:, 0:512].rearrange("p (s n) -> p s n", n=64),
                v[:, :, 0:64], v[:, :, 64:128], ALU.add)
            t2 = junk.tile([128, 1024], BF16, tag="junk", name=f"t2_{b}_{lc}")
            w = t1[:, 0:512].rearrange("p (s n) -> p s n", n=64)
            nc.vector.tensor_tensor(
                t2[:, 0:256].rearrange("p (s n) -> p s n", n=32),
                w[:, :, 0:32], w[:, :, 32:64], ALU.add)
            nc.vector.tensor_reduce(
                S_t[:, lc * 8:(lc + 1) * 8],
                t2[:, 0:256].rearrange("p (s n) -> p s n", n=32),
                axis=mybir.AxisListType.X, op=ALU.add,
            )
            Et_tiles.append(Et)

        # ---- g = tmw / (colsum + corr), column-doubled into matmul lhsT ----
        nc.vector.tensor_scalar(
            out=S_t[:], in0=S_t[:],
            scalar1=stcorrT[:, b:b + 1], scalar2=None, op0=ALU.add,
        )
        recipSt = small.tile([128, 64], FP32, tag="rSt")
        nc.vector.reciprocal(recipSt[:], S_t[:])
        g_all = small.tile([128, 64], FP32, tag="gall")
        nc.vector.tensor_tensor(g_all[:], recipSt[:], tmw64_t[b][:], ALU.mult)
        gdual_tiles = []
        for lc in range(8):
            gdual = gd_pool.tile([128, 512], BF16, tag="gd",
                                 name=f"gd_{b}_{lc}")
            nc.vector.tensor_copy(
                gdual[:].rearrange("p (h z) -> p h z", z=64),
                g_all[:, lc * 8:(lc + 1) * 8, None].to_broadcast((128, 8, 64)),
            )
            gdual_tiles.append(gdual)
        # w_t accumulation: valid quadrants (j<64, s=0) and (j>=64, s=1).
        # c-outer so each bank region is one contiguous start..stop group
        # (start=True clears has_written for the WHOLE bank).
        for c in range(NC4):
            for lc in range(8):
                nc.tensor.matmul(
                    ps_wt[:, c * 256:(c + 1) * 256],
                    lhsT=gdual_tiles[lc][:, 2 * c * 64:2 * c * 64 + 128],
                    rhs=Et_tiles[lc][:, c * 256:(c + 1) * 256],
                    start=(lc == 0), stop=(lc == 7),
                )

        # ---- ctx_t folds (two half-partition STTs per chunk) ----
        for c in range(NC4):
            scratch = junk.tile([128, 1024], BF16, tag="junk")
            nc.vector.scalar_tensor_tensor(
                out=scratch[0:64, 0:128],
                in0=ps_wt[0:64, c * 256:c * 256 + 128], scalar=1.0,
                in1=d2T_all[0:64, c * 512 + b * 128:c * 512 + (b + 1) * 128],
                op0=ALU.mult, op1=ALU.mult,
                accum_out=ctxv[0:64, 4 + c:5 + c],
            )
            nc.vector.scalar_tensor_tensor(
                out=scratch[64:128, 0:128],
                in0=ps_wt[64:128, c * 256 + 128:(c + 1) * 256], scalar=1.0,
                in1=d2T_all[64:128, c * 512 + b * 128:c * 512 + (b + 1) * 128],
                op0=ALU.mult, op1=ALU.mult,
                accum_out=ctxv[64:128, 4 + c:5 + c],
            )

        # ---------- transpose ctx [128, 8] -> [8, 128] and stage ----------
        ps_c = ps_acc.tile([128, 1024], FP32, tag="psa", name=f"ps_c_{b}")
        nc.tensor.transpose(ps_c[0:8, 0:128], ctxv[:], ident_f[:])
        nc.scalar.copy(ctxT_all[b * 32:b * 32 + 8, :], ps_c[0:8, 0:128])

        if nxt is not None:
            tT_cur = tT_nxt

    # ---------- output DMA: [32, 128] -> (BC, 1024) ----------
    for b in range(BC):
        nc.sync.dma_start(
            out=io["out"][b].rearrange("(j p) -> j p", j=8),
            in_=ctxT_all[b * 32:b * 32 + 8, :],
        )
    es.close()


def _build():
    nc = bacc.Bacc("TRN2", target_bir_lowering=False, debug=False,
                   num_devices=NCORES)
    io = {}

    def inp(name, shape, dt):
        io[name] = nc.dram_tensor(name, shape, dt, kind="ExternalInput").ap()

    inp("tgtT", [BC, KT, L], BF16)
    inp("drug_bf", [BC, KD, N], BF16)
    inp("wt", [KT, HID], BF16)
    inp("wd2", [KD, 2 * HID], BF16)
    inp("cpack", [128, 3 * BC + BC * 64], FP32)
    inp("rpack", [1, HID + 2 * HID + BC * L + BC * N], BF16)
    io["out"] = nc.dram_tensor("out", [BC, 2 * HID], FP32,
                               kind="ExternalOutput").ap()
    with tile.TileContext(nc) as tc:
        _body(tc, io)
    nc.compile()
    return nc


_NC_CACHE = None
_LAST_RESULTS = None


def _prep_host(drug_nodes, drug_mask, target_seq, target_mask,
               Wd, bd, Wt, bt, Wb):
    f32 = np.float32
    WdWb = np.einsum("khd,hde->khe", Wd.reshape(KD, H, HD), Wb).reshape(KD, HID)
    bdWb = np.einsum("hd,hde->he", bd.reshape(H, HD), Wb).reshape(HID)
    wd2 = np.ascontiguousarray(
        np.concatenate([Wd, WdWb], axis=1)).astype(BF16NP)
    wt_bf = np.ascontiguousarray(Wt).astype(BF16NP)
    b2 = np.concatenate([bd, bdWb]).astype(f32)
    dlen = np.maximum(drug_mask.sum(-1), 1).astype(f32)
    tlen = np.maximum(target_mask.sum(-1), 1).astype(f32)
    dmw = (drug_mask.astype(f32) / dlen[:, None]).astype(f32)
    tmw = (target_mask.astype(f32) / tlen[:, None]).astype(f32)
    sdcorr = (1e-30 - (L - tlen)).astype(f32)      # subtract invalid-l count
    stcorr = (1e-30 - (N - dlen)).astype(f32)      # subtract invalid-n count
    return wd2, wt_bf, b2, dmw, tmw, sdcorr, stcorr


def kernel(drug_nodes, drug_mask, target_seq, target_mask,
           Wd, bd, Wt, bt, Wb):
    f32 = np.float32
    drug_nodes = np.asarray(drug_nodes, f32)
    drug_mask = np.asarray(drug_mask)
    target_seq = np.asarray(target_seq, f32)
    target_mask = np.asarray(target_mask)
    Wd, bd = np.asarray(Wd, f32), np.asarray(bd, f32)
    Wt, bt = np.asarray(Wt, f32), np.asarray(bt, f32)
    Wb = np.asarray(Wb, f32)

    (wd2, wt_bf, b2, dmw, tmw, sdcorr, stcorr) = _prep_host(
        drug_nodes, drug_mask, target_seq, target_mask, Wd, bd, Wt, bt, Wb)

    tgt_bf = (target_seq * target_mask[:, :, None]).astype(BF16NP)
    tgtT_h = np.ascontiguousarray(tgt_bf.transpose(0, 2, 1))
    drug_bf = (drug_nodes * drug_mask[:, :, None]).astype(BF16NP)
    drugT_h = np.ascontiguousarray(drug_bf.transpose(0, 2, 1))
    bt_bf = bt.astype(BF16NP)
    b2_bf = b2.astype(BF16NP)
    tmask_bf = target_mask.astype(f32).astype(BF16NP)
    dmask_bf = drug_mask.astype(f32).astype(BF16NP)
    tmw64 = np.repeat(
        tmw.reshape(B, 8, 128).transpose(0, 2, 1), 8, axis=2)

    in_maps = []
    for i in range(NCORES):
        s = slice(i * BC, (i + 1) * BC)
        cpack = np.empty((128, 3 * BC + BC * 64), f32)
        cpack[:, 0:BC] = dmw[s].T
        cpack[:, BC:2 * BC] = sdcorr[s][None, :]
        cpack[:, 2 * BC:3 * BC] = stcorr[s][None, :]
        cpack[:, 3 * BC:] = tmw64[s].transpose(1, 0, 2).reshape(128, BC * 64)
        rpack = np.concatenate(
            [bt_bf, b2_bf, tmask_bf[s].ravel(), dmask_bf[s].ravel()])[None, :]
        in_maps.append(dict(
            tgtT=np.ascontiguousarray(tgtT_h[s]),
            drug_bf=np.ascontiguousarray(drugT_h[s]),
            wt=wt_bf, wd2=wd2,
            cpack=np.ascontiguousarray(cpack),
            rpack=np.ascontiguousarray(rpack),
        ))

    nc = _get_nc()
    res = run_bass_kernel_spmd(nc, in_maps, list(range(NCORES)))
    global _LAST_RESULTS
    _LAST_RESULTS = res
    out = np.concatenate([res.results[i]["out"] for i in range(NCORES)],
                         axis=0)
    return np.ascontiguousarray(out.astype(np.float32))


def _get_nc():
    global _NC_CACHE
    if _NC_CACHE is None:
        _NC_CACHE = _build()
    return _NC_CACHE


# revision 15
# speedup vs baseline: 1.0963x; 1.0224x over previous
"""BilinearAttention Trainium2 kernel — 8-core data-parallel (batch sharded).

Math per batch element b (reference semantics):
  d   = drug @ Wd + bd                     (N=128, HID=512)
  dWb = drug @ (Wd@Wb folded) + bdWb       (N, HID)     [host-folded weights]
  t   = target @ Wt + bt                   (L=1024, HID)
  per head h (HD=64):
    attn = dWb_h @ t_h^T                   (N, L)
    E    = exp(attn)  with masked rows/cols exactly 0 in the *inputs*
    d-side: p_d = E / rowsum(E);  w_d[l] = sum_n p_d * dm[n]/dlen
            ctx_d[h] = sum_l w_d[l] * t_h[l]
    t-side: p_t = E / colsum(E);  w_t[n] = sum_l p_t * tm[l]/tlen
            ctx_t[h] = sum_n w_t[n] * d_h[n]
  out[b] = [ctx_d(512) | ctx_t(512)]

Mask scheme: host zeroes invalid rows of drug/target; projection biases are
applied via rank-1 (bias x mask01) matmuls so projected features are exactly
0 at invalid positions.  exp(0)=1 at invalid attn entries; softmax
denominators are fixed by subtracting the host-known invalid counts.

Schedule notes:
 - prologue: drug projection for ALL 4 batch elements in one batched GEMM
   (starts ~2us in, warms the PE while tgtT(0) streams), then tproj(0)
   paced by per-k-chunk DMAs.
 - steady state: tproj(b+1) interleaved into the (ACT-limited) E phase of
   batch b so the tensor engine never drains.
 - t-side: attn^T recomputed per 128-row l-chunk with a head-pair repacked
   rhs (free=256 matmuls); the per-(l,head) softmax scale g = tmw/colsum is
   folded into the w_t reduction as the matmul *stationary* operand (gdual:
   [g_h | g_h'] column-doubled), so no broadcast multiply over Et is needed.
   The resulting PSUM has garbage in the cross quadrants, which are never
   read.
"""

import numpy as np
import ml_dtypes

import concourse.bass as bass
import concourse.bacc as bacc
import concourse.mybir as mybir
from concourse.bass_utils import run_bass_kernel_spmd
from concourse import tile
from concourse.masks import make_identity
from concourse.tile_rust import add_dep_helper

NCORES = 8
B = 32
BC = B // NCORES          # 4 batch elements per core
N, L = 128, 1024
KD, KT = 256, 1280        # drug dim, target dim
HID, H, HD = 512, 8, 64
NKC_T = KT // 128         # 10 k-chunks for target proj
NKC_D = KD // 128         # 2 k-chunks for drug proj
NC4 = HID // 128          # 4 hid chunks (2 heads each)
FP32 = mybir.dt.float32
BF16 = mybir.dt.bfloat16
AF = mybir.ActivationFunctionType
ALU = mybir.AluOpType
BF16NP = ml_dtypes.bfloat16


def _body(tc, io):
    nc = tc.nc
    import contextlib
    es = contextlib.ExitStack()

    const = es.enter_context(tc.tile_pool(name="const", bufs=1))

    # ---- constants / packed scalars (small, go first) ----
    # cpack fp32 [128, 3*BC + BC*64]: dmwT | sdcorrT | stcorrT | tmw64 per b
    cpack = const.tile([128, 3 * BC + BC * 64], FP32, tag="cpack")
    nc.sync.dma_start(out=cpack[:], in_=io["cpack"][:])
    dmwT = cpack[:, 0:BC]
    sdcorrT = cpack[:, BC:2 * BC]
    stcorrT = cpack[:, 2 * BC:3 * BC]
    tmw64_t = [cpack[:, 3 * BC + b * 64:3 * BC + (b + 1) * 64]
               for b in range(BC)]
    # rpack bf16 [1, 512 + 1024 + BC*1024 + BC*128]: bt | b2 | tmask | dmask
    rpack = const.tile([1, HID + 2 * HID + BC * L + BC * N], BF16, tag="rpack")
    nc.sync.dma_start(out=rpack[:], in_=io["rpack"][:])
    bt_row = rpack[:, 0:HID]
    b2_row = rpack[:, HID:HID + 2 * HID]
    tmask_row_t = [rpack[:, 3 * HID + b * L:3 * HID + (b + 1) * L]
                   for b in range(BC)]
    dmask_all = rpack[:, 3 * HID + BC * L:3 * HID + BC * L + BC * N]

    # drug (all 4 batches, [kd-part, (kc, b, n)]) + wd2 weights: small DMAs,
    # issued first so the drug projection can start ~2us in.
    drugT_all = const.tile([128, NKC_D * BC * N], BF16, tag="drugT")
    crit_dmas = []
    for kc in range(NKC_D):
        crit_dmas.append(nc.sync.dma_start(
            out=drugT_all[:, kc * BC * N:(kc + 1) * BC * N]
            .rearrange("p (b n) -> p b n", n=N),
            in_=io["drug_bf"][:, kc * 128:(kc + 1) * 128, :]
            .rearrange("b p n -> p b n"),
        ))
    wd2_all = const.tile([128, NKC_D * 2 * HID], BF16, tag="wd2all")
    crit_dmas.append(nc.sync.dma_start(
        out=wd2_all[:].rearrange("p (kc h) -> p kc h", h=2 * HID),
        in_=io["wd2"].rearrange("(kc p) h -> p kc h", p=128)))

    # target-proj weights: per-k-chunk DMAs so tproj(0) can stream.
    # Chunks beyond the first two are gated behind the (small) drug-side
    # loads so the drug projection can start ~2us in.
    wt_all = const.tile([128, NKC_T * HID], BF16, tag="wtall")
    for kc in range(NKC_T):
        i = nc.sync.dma_start(
            out=wt_all[:, kc * HID:(kc + 1) * HID],
            in_=io["wt"][kc * 128:(kc + 1) * 128, :])
        if kc >= 2:
            for cd in crit_dmas:
                add_dep_helper(i.ins, cd.ins, sync=True,
                               reason="bulk wt DMA after critical drug DMAs")

    ident_f = const.tile([128, 128], FP32, tag="idf")
    make_identity(nc, ident_f[:])
    ctxT_all = const.tile([128, 128], FP32, tag="ctxall")

    # d projections for all batches: d2T_all[:, ch*512 + b*128 + n]
    # ch 0..3 = d (value side), ch 4..7 = dW (bilinear-mapped, attn side)
    d2T_all = const.tile([128, 8 * BC * N], BF16, tag="d2Tall")
    # head-pair repacked dW with zero padding, per batch:
    # d2pair2[:, b*1024 + c*256 + s*128 + n]; valid rows s*64..s*64+63
    d2p2 = const.tile([128, BC * 2 * HID], BF16, tag="d2p2")

    # ---- pools ----
    tgtT_pool = es.enter_context(tc.tile_pool(name="tgtT", bufs=2))
    tT_pool = es.enter_context(tc.tile_pool(name="tT", bufs=2))
    e_pool = es.enter_context(tc.tile_pool(name="E", bufs=10))
    et_pool = es.enter_context(tc.tile_pool(name="Et", bufs=9))
    small = es.enter_context(tc.tile_pool(name="small", bufs=4))
    gd_pool = es.enter_context(tc.tile_pool(name="gd", bufs=9))
    junk = es.enter_context(tc.tile_pool(name="junk", bufs=3))
    ps_main = es.enter_context(tc.tile_pool(name="psm", bufs=3, space="PSUM"))
    ps_acc = es.enter_context(tc.tile_pool(name="psa", bufs=1, space="PSUM"))

    # ---------- helpers ----------
    def load_tgtT(bb, chunked=False):
        t = tgtT_pool.tile([128, NKC_T * L], BF16, tag="tgtT",
                           name=f"tgtT_{bb}")
        if chunked:
            for kc in range(NKC_T):
                i = nc.sync.dma_start(
                    out=t[:, kc * L:(kc + 1) * L],
                    in_=io["tgtT"][bb, kc * 128:(kc + 1) * 128, :])
                if kc >= 2:
                    for cd in crit_dmas:
                        add_dep_helper(
                            i.ins, cd.ins, sync=True,
                            reason="bulk tgt DMA after critical drug DMAs")
        else:
            for half in range(2):
                nc.sync.dma_start(
                    out=t[:].rearrange("p (kc l) -> p kc l", l=L)[
                        :, half * 5:(half + 1) * 5, :],
                    in_=io["tgtT"][bb, half * 5 * 128:(half + 1) * 5 * 128, :]
                    .rearrange("(kc p) l -> p kc l", p=128),
                )
        return t

    tproj_state = {}

    def tproj_group(bb, tgtT_t, slot):
        # slot 0..7 -> (c, lh); psum tile held across even/odd slot pairs
        c, lh = slot // 2, slot % 2
        if lh == 0:
            tproj_state["ps"] = ps_main.tile([128, 2 * 512], FP32, tag="psm",
                                             name=f"ps_tp_{bb}_{c}")
        ps = tproj_state["ps"]
        for kc in range(NKC_T):
            nc.tensor.matmul(
                ps[:, lh * 512:(lh + 1) * 512],
                lhsT=wt_all[:, kc * HID + c * 128:kc * HID + (c + 1) * 128],
                rhs=tgtT_t[:, kc * L + lh * 512:kc * L + (lh + 1) * 512],
                start=(kc == 0), stop=False,
            )
        nc.tensor.matmul(
            ps[:, lh * 512:(lh + 1) * 512],
            lhsT=bt_row[:, c * 128:(c + 1) * 128],
            rhs=tmask_row_t[bb][:, lh * 512:(lh + 1) * 512],
            start=False, stop=True,
        )
        if lh == 1:
            t = tT_pool.tile([128, L], BF16, tag=f"tT{c}", name=f"tT_{bb}_{c}")
            nc.scalar.copy(t[:], ps[:])
            return t
        return None

    # ---------- prologue ----------
    # drug projection, all 4 batches in one batched GEMM (free dim = b*n=512)
    for cp in range(4):            # ch pairs (0,1), (2,3), (4,5), (6,7)
        ps_d = ps_main.tile([128, 2 * 512], FP32, tag="psm",
                            name=f"ps_dp_{cp}")
        for half in range(2):
            ch = 2 * cp + half
            for kc in range(NKC_D):
                nc.tensor.matmul(
                    ps_d[:, half * 512:(half + 1) * 512],
                    lhsT=wd2_all[:, kc * 2 * HID + ch * 128:
                                 kc * 2 * HID + (ch + 1) * 128],
                    rhs=drugT_all[:, kc * 512:(kc + 1) * 512],
                    start=(kc == 0), stop=False,
                )
            nc.tensor.matmul(
                ps_d[:, half * 512:(half + 1) * 512],
                lhsT=b2_row[:, ch * 128:(ch + 1) * 128],
                rhs=dmask_all[:],
                start=False, stop=True,
            )
        nc.scalar.copy(d2T_all[:, cp * 1024:(cp + 1) * 1024], ps_d[:])

    # head-pair repacked dW (zero padded halves)
    nc.vector.memset(d2p2[:], 0.0)
    for s in range(2):
        for bb in range(BC):
            nc.vector.tensor_copy(
                d2p2[s * 64:(s + 1) * 64, bb * 1024:(bb + 1) * 1024]
                .rearrange("q (c z) -> q c z", z=256)
                [:, :, s * 128:(s + 1) * 128],
                d2T_all[s * 64:(s + 1) * 64, 4 * BC * N:8 * BC * N]
                .rearrange("q (c z) -> q c z", z=512)
                [:, :, bb * 128:(bb + 1) * 128],
            )

    # tproj(0): streamed against the per-chunk DMAs
    tgtT_cur = load_tgtT(0, chunked=True)
    tT_cur = []
    for slot in range(8):
        t = tproj_group(0, tgtT_cur, slot)
        if t is not None:
            tT_cur.append(t)

    # ---------- per-batch steady state ----------
    for b in range(BC):
        nxt = b + 1 if b + 1 < BC else None
        if nxt is not None:
            tgtT_nxt = load_tgtT(nxt)
        tT = tT_cur

        # ---- E phase (ACT-limited) interleaved with t-proj(b+1) c=0,1 ----
        E = [e_pool.tile([128, L], BF16, tag="E", name=f"E_{b}_{i}")
             for i in range(H)]
        S_d8 = small.tile([128, 8], FP32, tag="Sd8")
        tT_nxt = []
        for h in range(H):
            c, ph = h // 2, (h % 2) * 64
            ps = ps_main.tile([128, 2 * 512], FP32, tag="psm",
                              name=f"ps_E_{b}_{h}")
            for lh in range(2):
                nc.tensor.matmul(
                    ps[:, lh * 512:(lh + 1) * 512],
                    lhsT=d2T_all[ph:ph + 64,
                                 (4 + c) * 512 + b * 128:(4 + c) * 512 + (b + 1) * 128],
                    rhs=tT[c][ph:ph + 64, lh * 512:(lh + 1) * 512],
                    start=True, stop=True,
                )
            nc.scalar.activation(
                E[h][:], ps[:], AF.Exp,
                accum_out=S_d8[:, h:h + 1],
            )
            if nxt is not None and h % 2 == 0:
                t = tproj_group(nxt, tgtT_nxt, h // 2)
                if t is not None:
                    tT_nxt.append(t)

        # ---- u pipeline (DVE) ----
        nc.vector.tensor_scalar(
            out=S_d8[:], in0=S_d8[:], scalar1=sdcorrT[:, b:b + 1],
            scalar2=None, op0=ALU.add,
        )
        recipSd = small.tile([128, 8], FP32, tag="rSd")
        nc.vector.reciprocal(recipSd[:], S_d8[:])
        u_f = small.tile([128, 8], FP32, tag="uf")
        nc.vector.tensor_scalar(
            out=u_f[:], in0=recipSd[:], scalar1=dmwT[:, b:b + 1],
            scalar2=None, op0=ALU.mult,
        )
        u_rep = small.tile([128, 8 * 64], BF16, tag="urep")
        nc.vector.tensor_copy(
            u_rep[:].rearrange("p (h z) -> p h z", z=64),
            u_f[:, :, None].to_broadcast((128, 8, 64)),
        )

        # ---- Et phase: attn^T per l-chunk; g folded into the w_t matmul.
        # w_d groups, tproj(b+1) c=2,3, and the per-half g chain are all
        # interleaved into this loop.
        ctxv = small.tile([128, 8], FP32, tag="ctx")
        S_t = small.tile([128, 64], FP32, tag="St")
        ps_wt = ps_acc.tile([128, 1024], FP32, tag="psa", name=f"ps_wt_{b}")
        Et_tiles = []
        mm_bank_start = {}

        def wd_group(c):
            ps = ps_main.tile([128, 2 * 512], FP32, tag="psm",
                              name=f"ps_wd_{b}_{c}")
            for lh in range(2):
                for hp in range(2):
                    h = 2 * c + hp
                    nc.tensor.matmul(
                        ps[hp * 64:(hp + 1) * 64, lh * 512:(lh + 1) * 512],
                        lhsT=u_rep[:, h * 64:(h + 1) * 64],
                        rhs=E[h][:, lh * 512:(lh + 1) * 512],
                        start=True, stop=True,
                    )
            scratch = junk.tile([128, 1024], BF16, tag="junk")
            nc.vector.scalar_tensor_tensor(
                out=scratch[:], in0=ps[:], scalar=1.0,
                in1=tT[c][:],
                op0=ALU.mult, op1=ALU.mult,
                accum_out=ctxv[:, c:c + 1],
            )

        for lc in range(8):
            ps = ps_main.tile([128, 2 * 512], FP32, tag="psm",
                              name=f"ps_Et_{b}_{lc}")
            for c in range(NC4):
                nc.tensor.matmul(
                    ps[:, c * 256:(c + 1) * 256],
                    lhsT=tT[c][:, lc * 128:(lc + 1) * 128],
                    rhs=d2p2[:, b * 1024 + c * 256:b * 1024 + (c + 1) * 256],
                    start=True, stop=True,
                )
            Et = et_pool.tile([128, 1024], BF16, tag="Et", name=f"Et_{b}_{lc}")
            nc.scalar.activation(Et[:], ps[:], AF.Exp)
            # colsum: pairwise add on GpSimd (otherwise idle), reduce on DVE
            t1 = junk.tile([128, 1024], BF16, tag="junk", name=f"t1_{b}_{lc}")
            v = Et[:].rearrange("p (s n) -> p s n", n=128)
            nc.gpsimd.tensor_tensor(
                t1[:, 0:512].rearrange("p (s n) -> p s n", n=64),
                v[:, :, 0:64], v[:, :, 64:128], ALU.add)
            nc.vector.tensor_reduce(
                S_t[:, lc * 8:(lc + 1) * 8],
                t1[:, 0:512].rearrange("p (s n) -> p s n", n=64),
                axis=mybir.AxisListType.X, op=ALU.add,
            )
            Et_tiles.append(Et)

            if lc % 2 == 0:
                wd_group(lc // 2)
            else:
                if nxt is not None:
                    t = tproj_group(nxt, tgtT_nxt, 4 + lc // 2)
                    if t is not None:
                        tT_nxt.append(t)

            if lc % 4 == 3:
                # g = tmw / (colsum + corr) for this half of the l-chunks,
                # column-doubled into the w_t matmul's stationary operand
                half = lc // 4
                sl = slice(half * 32, half * 32 + 32)
                nc.vector.tensor_scalar(
                    out=S_t[:, sl], in0=S_t[:, sl],
                    scalar1=stcorrT[:, b:b + 1], scalar2=None, op0=ALU.add,
                )
                recipSt = small.tile([128, 32], FP32, tag="rSt",
                                     name=f"rSt_{b}_{half}")
                nc.vector.reciprocal(recipSt[:], S_t[:, sl])
                g_half = small.tile([128, 32], FP32, tag="gh",
                                    name=f"gh_{b}_{half}")
                nc.vector.tensor_tensor(
                    g_half[:], recipSt[:], tmw64_t[b][:, sl], ALU.mult)
                for lcc in range(half * 4, half * 4 + 4):
                    gdual = gd_pool.tile([128, 512], BF16, tag="gd",
                                         name=f"gd_{b}_{lcc}")
                    nc.vector.tensor_copy(
                        gdual[:].rearrange("p (h z) -> p h z", z=64),
                        g_half[:, (lcc % 4) * 8:(lcc % 4) * 8 + 8, None]
                        .to_broadcast((128, 8, 64)),
                    )
                    # w_t accumulation: valid quadrants (j<64, s=0) and
                    # (j>=64, s=1). start=True only on the first MM touching
                    # each bank (it clears has_written bank-wide); all other
                    # MMs are ordered after it explicitly.
                    for c in range(NC4):
                        st = (lcc == 0 and c % 2 == 0)
                        sp = (lcc == 7 and c % 2 == 1)
                        mm = nc.tensor.matmul(
                            ps_wt[:, c * 256:(c + 1) * 256],
                            lhsT=gdual[:, 2 * c * 64:2 * c * 64 + 128],
                            rhs=Et_tiles[lcc][:, c * 256:(c + 1) * 256],
                            start=st, stop=sp, skip_group_check=True,
                        )
                        bank = c // 2
                        if st:
                            mm_bank_start[bank] = mm
                        else:
                            add_dep_helper(
                                mm.ins, mm_bank_start[bank].ins, sync=False,
                                reason="ps_wt accum after bank-clearing MM")

        # ---- ctx_t folds (two half-partition STTs per chunk) ----
        for c in range(NC4):
            scratch = junk.tile([128, 1024], BF16, tag="junk")
            nc.vector.scalar_tensor_tensor(
                out=scratch[0:64, 0:128],
                in0=ps_wt[0:64, c * 256:c * 256 + 128], scalar=1.0,
                in1=d2T_all[0:64, c * 512 + b * 128:c * 512 + (b + 1) * 128],
                op0=ALU.mult, op1=ALU.mult,
                accum_out=ctxv[0:64, 4 + c:5 + c],
            )
            nc.vector.scalar_tensor_tensor(
                out=scratch[64:128, 0:128],
                in0=ps_wt[64:128, c * 256 + 128:(c + 1) * 256], scalar=1.0,
                in1=d2T_all[64:128, c * 512 + b * 128:c * 512 + (b + 1) * 128],
                op0=ALU.mult, op1=ALU.mult,
                accum_out=ctxv[64:128, 4 + c:5 + c],
            )

        # ---------- transpose ctx [128, 8] -> [8, 128] and ship ----------
        ps_c = ps_acc.tile([128, 1024], FP32, tag="psa", name=f"ps_c_{b}")
        nc.tensor.transpose(ps_c[0:8, 0:128], ctxv[:], ident_f[:])
        nc.scalar.copy(ctxT_all[b * 32:b * 32 + 8, :], ps_c[0:8, 0:128])
        nc.sync.dma_start(
            out=io["out"][b].rearrange("(j p) -> j p", j=8),
            in_=ctxT_all[b * 32:b * 32 + 8, :],
        )

        if nxt is not None:
            tT_cur = tT_nxt
    es.close()


def _build():
    nc = bacc.Bacc("TRN2", target_bir_lowering=False, debug=False,
                   num_devices=NCORES)
    io = {}

    def inp(name, shape, dt):
        io[name] = nc.dram_tensor(name, shape, dt, kind="ExternalInput").ap()

    inp("tgtT", [BC, KT, L], BF16)
    inp("drug_bf", [BC, KD, N], BF16)
    inp("wt", [KT, HID], BF16)
    inp("wd2", [KD, 2 * HID], BF16)
    inp("cpack", [128, 3 * BC + BC * 64], FP32)
    inp("rpack", [1, HID + 2 * HID + BC * L + BC * N], BF16)
    io["out"] = nc.dram_tensor("out", [BC, 2 * HID], FP32,
                               kind="ExternalOutput").ap()
    with tile.TileContext(nc) as tc:
        _body(tc, io)
    nc.compile()
    return nc


_NC_CACHE = None
_LAST_RESULTS = None


def _prep_host(drug_nodes, drug_mask, target_seq, target_mask,
               Wd, bd, Wt, bt, Wb):
    f32 = np.float32
    WdWb = np.einsum("khd,hde->khe", Wd.reshape(KD, H, HD), Wb).reshape(KD, HID)
    bdWb = np.einsum("hd,hde->he", bd.reshape(H, HD), Wb).reshape(HID)
    wd2 = np.ascontiguousarray(
        np.concatenate([Wd, WdWb], axis=1)).astype(BF16NP)
    wt_bf = np.ascontiguousarray(Wt).astype(BF16NP)
    b2 = np.concatenate([bd, bdWb]).astype(f32)
    dlen = np.maximum(drug_mask.sum(-1), 1).astype(f32)
    tlen = np.maximum(target_mask.sum(-1), 1).astype(f32)
    dmw = (drug_mask.astype(f32) / dlen[:, None]).astype(f32)
    tmw = (target_mask.astype(f32) / tlen[:, None]).astype(f32)
    sdcorr = (1e-30 - (L - tlen)).astype(f32)      # subtract invalid-l count
    stcorr = (1e-30 - (N - dlen)).astype(f32)      # subtract invalid-n count
    return wd2, wt_bf, b2, dmw, tmw, sdcorr, stcorr


def kernel(drug_nodes, drug_mask, target_seq, target_mask,
           Wd, bd, Wt, bt, Wb):
    f32 = np.float32
    drug_nodes = np.asarray(drug_nodes, f32)
    drug_mask = np.asarray(drug_mask)
    target_seq = np.asarray(target_seq, f32)
    target_mask = np.asarray(target_mask)
    Wd, bd = np.asarray(Wd, f32), np.asarray(bd, f32)
    Wt, bt = np.asarray(Wt, f32), np.asarray(bt, f32)
    Wb = np.asarray(Wb, f32)

    (wd2, wt_bf, b2, dmw, tmw, sdcorr, stcorr) = _prep_host(
        drug_nodes, drug_mask, target_seq, target_mask, Wd, bd, Wt, bt, Wb)

    tgt_bf = (target_seq * target_mask[:, :, None]).astype(BF16NP)
    tgtT_h = np.ascontiguousarray(tgt_bf.transpose(0, 2, 1))
    drug_bf = (drug_nodes * drug_mask[:, :, None]).astype(BF16NP)
    drugT_h = np.ascontiguousarray(drug_bf.transpose(0, 2, 1))
    bt_bf = bt.astype(BF16NP)
    b2_bf = b2.astype(BF16NP)
    tmask_bf = target_mask.astype(f32).astype(BF16NP)
    dmask_bf = drug_mask.astype(f32).astype(BF16NP)
    tmw64 = np.repeat(
        tmw.reshape(B, 8, 128).transpose(0, 2, 1), 8, axis=2)

    in_maps = []
    for i in range(NCORES):
        s = slice(i * BC, (i + 1) * BC)
        cpack = np.empty((128, 3 * BC + BC * 64), f32)
        cpack[:, 0:BC] = dmw[s].T
        cpack[:, BC:2 * BC] = sdcorr[s][None, :]
        cpack[:, 2 * BC:3 * BC] = stcorr[s][None, :]
        cpack[:, 3 * BC:] = tmw64[s].transpose(1, 0, 2).reshape(128, BC * 64)
        rpack = np.concatenate(
            [bt_bf, b2_bf, tmask_bf[s].ravel(), dmask_bf[s].ravel()])[None, :]
        in_maps.append(dict(
            tgtT=np.ascontiguousarray(tgtT_h[s]),
            drug_bf=np.ascontiguousarray(drugT_h[s]),
            wt=wt_bf, wd2=wd2,
            cpack=np.ascontiguousarray(cpack),
            rpack=np.ascontiguousarray(rpack),
        ))

    nc = _get_nc()
    res = run_bass_kernel_spmd(nc, in_maps, list(range(NCORES)))
    global _LAST_RESULTS
    _LAST_RESULTS = res
    out = np.concatenate([res.results[i]["out"] for i in range(NCORES)],
                         axis=0)
    return np.ascontiguousarray(out.astype(np.float32))


def _get_nc():
    global _NC_CACHE
    if _NC_CACHE is None:
        _NC_CACHE = _build()
    return _NC_CACHE
